# revision 50
# baseline (speedup 1.0000x reference)
"""Attention-LSTM captioning model on 8 trn2 cores (8-way tensor parallel).

Gate/itr/vocab output dims sharded across cores (full B=64 per core);
attention batch-sharded (8 batches/core, selected via per-core one-hot bsel
data, since the SPMD program is identical on every core). Activations are
transposed [feature, batch]. Per step: AllGather(att_resT + piggybacked
log-softmax stats), AllGather(nh chunk). Sigmoid(x) is computed as
(tanh(x/2)+1)/2 so the step only needs the {tanh, exp} ACT table; the hidden
state is stored as 2*h with h-consuming weights pre-halved on the host.

Per-step schedule fills both AllGather stall windows: the logit matmul for
step t-1, the gates matmul for step t, per-step log-softmax stats (bf16-
rounded max trick so the stats collapse to 2 bf16 scalars per row riding the
next AG_A), and the final logZ-subtract + output DMA for step t-2.
"""
import numpy as np
import ml_dtypes

import concourse.bacc as bacc
import concourse.mybir as mybir
import concourse.tile as tile
from concourse.ap import AP
from concourse.bass_utils import run_bass_kernel_spmd

BF16_NP = ml_dtypes.bfloat16
FP32 = mybir.dt.float32
BF16 = mybir.dt.bfloat16
AF = mybir.ActivationFunctionType
ALU = mybir.AluOpType
AX = mybir.AxisListType

B, T, R, H, F, E, L, V1 = 64, 20, 1024, 512, 2048, 300, 196, 12001
NC = 8
BMY = B // NC
GC = R // NC              # 128
NGATE = 5 * GC            # 640
VP = 1504
LP = 224
NG = LP // 16             # 14
NGP = NG // 2             # 7 fp8 DoubleRow pairs
EP = 384
HCN = H // 128            # 4
FCN = F // 128            # 16
RCN = R // 128            # 8
NBL = BMY * L             # 1568
FP8 = mybir.dt.float8e4
F8_NP = ml_dtypes.float8_e4m3
LOGIT_SCALE = 32.0        # fp8 range prescale on logit weights
H2ATT_SCALE = 128.0       # fp8 range prescale on h2att weights
CTX_SCALE = 64.0          # fp8 range prescale on ctx2att weights
LN16 = 2.772588722239781  # attention weights scaled by 16 for fp8 range


def _bf(x):
    return np.ascontiguousarray(np.asarray(x, dtype=np.float32)).astype(BF16_NP)


def _f8(x):
    return np.ascontiguousarray(np.asarray(x, dtype=np.float32)).astype(F8_NP)


def bcast_free(ap, n):
    """Append a step-0 free dim of size n to an AP (broadcast)."""
    return AP(ap.tensor, ap.offset, list(ap.ap) + [[0, n]])


def host_prep(inputs):
    seq = np.asarray(inputs["seq"])
    att = np.asarray(inputs["att_feats"], dtype=np.float32)
    embed_w = np.asarray(inputs["embed_w"], dtype=np.float32)
    ctx2att_w = np.asarray(inputs["ctx2att_w"], dtype=np.float32)
    ctx2att_b = np.asarray(inputs["ctx2att_b"], dtype=np.float32)
    h2att_w = np.asarray(inputs["h2att_w"], dtype=np.float32)
    h2att_b = np.asarray(inputs["h2att_b"], dtype=np.float32)
    alpha_w = np.asarray(inputs["alpha_w"], dtype=np.float32)
    i2h_w = np.asarray(inputs["i2h_w"], dtype=np.float32)
    i2h_b = np.asarray(inputs["i2h_b"], dtype=np.float32)
    h2h_w = np.asarray(inputs["h2h_w"], dtype=np.float32)
    h2h_b = np.asarray(inputs["h2h_b"], dtype=np.float32)
    a2c_w = np.asarray(inputs["a2c_w"], dtype=np.float32)
    a2c_b = np.asarray(inputs["a2c_b"], dtype=np.float32)
    logit_w = np.asarray(inputs["logit_w"], dtype=np.float32)
    logit_b = np.asarray(inputs["logit_b"], dtype=np.float32)

    xt = embed_w[seq]                                    # [B, T, E]
    # stream-major batch order: col t*64 + s*32 + (c*4 + j)  <->  global
    # batch c*8 + s*4 + j  (stream s = local batches s*4..s*4+4 of each core)
    gb = np.zeros(B, dtype=np.int64)
    for s in range(2):
        for c in range(NC):
            for j in range(4):
                gb[s * 32 + c * 4 + j] = c * 8 + s * 4 + j
    xtr = xt[gb]                                         # [B(reord), T, E]
    xtT = np.zeros((EP, T * B), dtype=np.float32)
    xtT[:E] = xtr.transpose(2, 1, 0).reshape(E, T * B)
    xtT[E] = 1.0
    xtT = _bf(xtT)
    bias_gate = i2h_b + h2h_b

    in_maps = []
    for c in range(NC):
        m = {"xtT": xtT}
        grows = np.concatenate([np.arange(gg * R + c * GC, gg * R + (c + 1) * GC)
                                for gg in range(5)])
        i2hT = np.zeros((EP, NGATE), dtype=np.float32)
        i2hT[:E] = i2h_w[grows, :].T
        i2hT[E] = bias_gate[grows]
        # fold the a2c bias into the itr-part gate bias (itr = sums + ctx)
        arows_b = np.concatenate([np.arange(c * GC, (c + 1) * GC),
                                  np.arange(R + c * GC, R + (c + 1) * GC)])
        i2hT[E, 384:640] += a2c_b[arows_b]
        m["i2hT"] = _bf(i2hT)
        m["h2hT"] = _bf(h2h_w[grows, :].T * 0.5)
        h2a = (h2att_w.T * (0.5 * H2ATT_SCALE)).reshape(RCN, 128, H)
        m["h2att8"] = _f8(np.concatenate([h2a[0::2], h2a[1::2]],
                                         axis=2).reshape(RCN // 2 * 128,
                                                         2 * H))
        cx = (ctx2att_w.T * CTX_SCALE).reshape(FCN, 128, H)
        m["ctxT8"] = _f8(np.concatenate([cx[0::2], cx[1::2]],
                                        axis=2).reshape(FCN // 2 * 128,
                                                        2 * H))
        # h2att_bias folded in: dot = tanh(p_att + h@h2attT) with both
        # biases additive per-h element (pre-scaled: pa psum is CTX_SCALE x)
        m["ctx_bias"] = _bf((ctx2att_b + h2att_b)[None, :] * CTX_SCALE)
        amy = att[c * BMY:(c + 1) * BMY]                 # [8, L, F]
        at8 = amy.transpose(2, 0, 1).reshape(FCN, 128, NBL)
        m["attT8"] = _f8(np.concatenate([at8[0::2], at8[1::2]],
                                        axis=2).reshape(FCN // 2 * 128,
                                                        2 * NBL))
        # partition = s*64 + lw*4 + b4  (stream s, local batch b = s*4+b4)
        alb = np.zeros((NG, 16, 2, 4, F), dtype=np.float32)
        for l in range(L):
            alb[l // 16, l % 16, :, :, :] = amy[:, l, :].reshape(2, 4, F)
        alb = alb.transpose(0, 2, 1, 3, 4).reshape(NG, 128, F)
        # fp8 DoubleRow pair packing: tile j holds groups (2j | 2j+1)
        m["att_lb8"] = _f8(np.concatenate([alb[0::2], alb[1::2]],
                                          axis=2).reshape(NGP * 128, 2 * F))
        ac = np.zeros((HCN * 128, 32), dtype=np.float32)
        for s in range(2):
            for b4 in range(4):
                ac[:, s * 16 + b4 * 4 + b4] = alpha_w[0]
        m["alpha_cols"] = _bf(ac)
        arows = np.concatenate([np.arange(c * GC, (c + 1) * GC),
                                np.arange(R + c * GC, R + (c + 1) * GC)])
        m["a2cT"] = _bf(a2c_w[arows, :].T)
        vrows = np.arange(c * VP, (c + 1) * VP)
        lw = np.zeros((R, VP), dtype=np.float32)
        lb = np.full((1, VP), -1e30, dtype=np.float32)
        valid = vrows < V1
        lw[:, valid] = logit_w[vrows[valid], :].T * 0.5
        lb[0, valid] = logit_b[vrows[valid]]
        lwr = (lw * LOGIT_SCALE).reshape(RCN, 128, VP)
        m["logit8"] = _f8(np.concatenate([lwr[0::2], lwr[1::2]],
                                         axis=2).reshape(RCN // 2 * 128,
                                                         2 * VP))
        m["logit_bias"] = lb
        m["ident"] = _bf(np.eye(128))
        # per-stream one-hot: row s*32 + (c'*4 + j) -> col j for c' == c
        bsel = np.zeros((64, 4), dtype=np.float32)
        for s in range(2):
            for j in range(4):
                bsel[s * 32 + c * 4 + j, j] = 1.0
        m["bsel"] = _bf(bsel)
        in_maps.append(m)
    return in_maps


def build(t_steps=T, probes=(), reps=1, no_cc=False):
    nc = bacc.Bacc("TRN2", target_bir_lowering=False, debug=False,
                   num_devices=NC)
    probes = set(probes)
    RG = [list(range(NC))]
    AGW = 66    # per-stream agA payload: 64 arT + 2 stat cols

    def din(name, shape, dt=BF16):
        return nc.dram_tensor(name, shape, dt, kind="ExternalInput")

    xtT_d = din("xtT", [EP, T * B])
    i2hT_d = din("i2hT", [EP, NGATE])
    h2hT_d = din("h2hT", [R, NGATE])
    h2att8_d = din("h2att8", [RCN // 2 * 128, 2 * H], FP8)
    ctxT8_d = din("ctxT8", [FCN // 2 * 128, 2 * H], FP8)
    ctx_b_d = din("ctx_bias", [1, H])
    attT8_d = din("attT8", [FCN // 2 * 128, 2 * NBL], FP8)
    att_lb8_d = din("att_lb8", [NGP * 128, 2 * F], FP8)
    alpha_d = din("alpha_cols", [HCN * 128, 32])
    a2cT_d = din("a2cT", [F, 256])
    logit8_d = din("logit8", [RCN // 2 * 128, 2 * VP], FP8)
    logit_b_d = din("logit_bias", [1, VP], FP32)
    ident_d = din("ident", [128, 128])
    bsel_d = din("bsel", [64, 4])

    out_d = nc.dram_tensor("logp", [t_steps * B, VP], FP32,
                           kind="ExternalOutput")
    agA_out_r = [[[nc.dram_tensor(f"agA_out_{rp}_{s}_{t}",
                                  [NC * 128, AGW], BF16,
                                  addr_space="Shared")
                   for t in range(t_steps)] for s in range(2)]
                 for rp in range(reps)]
    agH_out_r = [[[nc.dram_tensor(f"agH_out_{rp}_{s}_{t}", [R, 32], BF16,
                                  addr_space="Shared")
                   for t in range(t_steps)] for s in range(2)]
                 for rp in range(reps)]
    agS_out_r = [nc.dram_tensor(f"agS_out_{rp}", [NC * 64, 4], BF16,
                                addr_space="Shared") for rp in range(reps)]

    with tile.TileContext(nc) as tc:
        with (
            tc.tile_pool(name="wpool", bufs=1) as wpool,
            tc.tile_pool(name="hpool", bufs=4) as hpool,
            tc.tile_pool(name="psum", bufs=1, space="PSUM") as psum,
            tc.tile_pool(name="dram", bufs=4, space="DRAM") as dpool,
        ):
            def probe_(name, src_ap, shape, dt):
                pd = nc.dram_tensor(f"probe_{name}", list(shape), dt,
                                    kind="ExternalOutput")
                nc.sync.dma_start(out=pd[:], in_=src_ap)

            def load_chunks(pool, dram, cols, n, tag, dt=BF16):
                ts = []
                for i in range(n):
                    t_ = pool.tile([128, cols], dt, tag=f"{tag}{i}",
                                   name=f"{tag}{i}")
                    nc.sync.dma_start(out=t_[:],
                                      in_=dram[i * 128:(i + 1) * 128, :])
                    ts.append(t_)
                return ts

            logit8_s = load_chunks(wpool, logit8_d, 2 * VP, RCN // 2,
                                   "logit8", dt=FP8)
            logit_b_s = wpool.tile([64, VP], FP32, tag="logitb",
                                   name="logitb")
            _lb_src = AP(logit_b_d[:].tensor, logit_b_d[:].offset,
                         [[0, 64], [1, VP]])
            nc.sync.dma_start(out=logit_b_s[:], in_=_lb_src)
            ident_s = wpool.tile([128, 128], BF16, tag="ident", name="ident")
            nc.sync.dma_start(out=ident_s[:], in_=ident_d[:])
            ones64 = wpool.tile([1, B], BF16, tag="ones64", name="ones64")
            nc.vector.memset(ones64[:], 1.0)

            with tc.tile_pool(name="w1pool", bufs=1) as w1pool:
                xtT_s = load_chunks(w1pool, xtT_d, T * B, 3, "xtT")
                i2hT_s = load_chunks(w1pool, i2hT_d, NGATE, 3, "i2hT")
                h2hT_s = load_chunks(w1pool, h2hT_d, NGATE, RCN, "h2hT")
                h2att8_s = load_chunks(w1pool, h2att8_d, 2 * H, RCN // 2,
                                       "h2att8", dt=FP8)
                # shared tiles; stream s uses partitions s*64:(s+1)*64
                att8_t = load_chunks(w1pool, att_lb8_d, 2 * F, NGP,
                                     "attlb8", dt=FP8)
                alpha_s = load_chunks(w1pool, alpha_d, 32, HCN, "alpha")
                a2cT_s = load_chunks(w1pool, a2cT_d, 256, FCN, "a2cT")
                bsel_s = []
                for s in range(2):
                    t_ = w1pool.tile([32, 4], BF16, tag=f"bsel{s}",
                                     name=f"bsel{s}")
                    nc.sync.dma_start(out=t_[:],
                                      in_=bsel_d[s * 32:(s + 1) * 32, :])
                    bsel_s.append(t_)
                ctx_b_s = w1pool.tile([1, H], BF16, tag="ctxb", name="ctxb")
                nc.sync.dma_start(out=ctx_b_s[:], in_=ctx_b_d[:])
                onesNBL = w1pool.tile([1, NBL], BF16, tag="onesNBL",
                                      name="onesNBL")
                nc.vector.memset(onesNBL[:], 1.0)
                p_attT = [w1pool.tile([128, NBL], BF16, tag=f"pattT{hc}",
                                      name=f"pattT{hc}")
                          for hc in range(HCN)]
                # col = g2*32 + gp*16 + b4: DoubleRow pair (gp) stride 16
                # (needs step%16==0) and the scatter dest merges to 2 dims;
                # partitions s*64 + lw*4 + b4 (stream-sliced)
                stat_all = w1pool.tile([128, LP], FP8, tag="stat_all",
                                       name="stat_all")
                nc.vector.memset(stat_all[:], 0.0)
                w_bf = [w1pool.tile([4, LP], FP8, tag=f"w_bf{s}",
                                    name=f"w_bf{s}") for s in range(2)]
                c_st = [w1pool.tile([32, GC], FP32, tag=f"c_st{s}",
                                    name=f"c_st{s}") for s in range(2)]
                statb = [w1pool.tile([32, 4], BF16, tag=f"statb{s}",
                                     name=f"statb{s}") for s in range(2)]
                for s in range(2):
                    nc.vector.memset(w_bf[s][:], 0.0)

                def emit_rep(rep):
                    agA_out = agA_out_r[rep]
                    agH_out = agH_out_r[rep]
                    agS_out = agS_out_r[rep]

                    ST = []
                    for s in range(2):
                        nc.vector.memset(c_st[s][:], 0.0)
                        nc.vector.memset(statb[s][:], 0.0)
                        hT0 = hpool.tile([128, RCN * 32], BF16,
                                         tag=f"hT{s}", name=f"hT0_{s}")
                        nc.vector.memset(hT0[:], 0.0)
                        h80 = hpool.tile([128, RCN * 32], FP8,
                                         tag=f"hT8{s}", name=f"hT80_{s}")
                        nc.vector.memset(h80[:], 0.0)
                        ST.append({"hT": [hT0], "hT8": [h80], "lg": {},
                                   "nZ": {}, "sig3": {}, "sitr": {}})

                    # ---------- phase 0 (fp8 DoubleRow over fc pairs) ------
                    with (
                        tc.tile_pool(name=f"ctxpool{rep}", bufs=1) as ctxpool,
                        tc.tile_pool(name=f"stream{rep}", bufs=3) as stream,
                    ):
                        ctxT8_s = load_chunks(ctxpool, ctxT8_d, 2 * H,
                                              FCN // 2, "ctxT8", dt=FP8)
                        QW = 392
                        for qq in range(4):
                            n0 = qq * QW
                            _pa_tags = [("sums", 1), ("mid", 2), ("ar", 1),
                                        ("small", 2)]
                            pa_ps = [psum.tile([128, QW], FP32,
                                               tag=_pa_tags[hc][0],
                                               name=f"pa{hc}",
                                               bufs=_pa_tags[hc][1])
                                     for hc in range(HCN)]
                            for jf in range(FCN // 2):
                                at = stream.tile([128, 2 * QW], FP8,
                                                 tag="attTq", name="attTq")
                                a8 = attT8_d[jf * 128:(jf + 1) * 128, 0:1]
                                nc.sync.dma_start(
                                    out=at[:],
                                    in_=AP(a8.tensor, a8.offset + n0,
                                           [a8.ap[0], [NBL, 2], [1, QW]]))
                                for hc in range(HCN):
                                    cx = ctxT8_s[jf][:, 0:1]
                                    lhsT = AP(cx.tensor,
                                              cx.offset + hc * 128,
                                              [cx.ap[0], [H, 2], [1, 128]])
                                    rhs = at[:].rearrange(
                                        "p (two q) -> p two q", two=2)
                                    nc.tensor.matmul(
                                        pa_ps[hc][:], lhsT, rhs,
                                        perf_mode=(mybir.MatmulPerfMode
                                                   .DoubleRow),
                                        start=(jf == 0), stop=False)
                            for hc in range(HCN):
                                nc.tensor.matmul(
                                    pa_ps[hc][:],
                                    ctx_b_s[:, hc * 128:(hc + 1) * 128],
                                    onesNBL[:, n0:n0 + QW], start=False,
                                    stop=True)
                                nc.vector.tensor_scalar(
                                    p_attT[hc][:, n0:n0 + QW], pa_ps[hc][:],
                                    1.0 / CTX_SCALE, None, op0=ALU.mult)

                    # ---------- phase 1: slot-pipelined 2 streams ----------
                    with tc.tile_pool(name=f"work1_{rep}", bufs=1) as work:

                        def logit_mms(s, ts, c0, c1, gate=None):
                            lg_ps = psum.tile([32, 512], FP32, tag="lg",
                                              name=f"lg_{s}_{ts}_{c0}",
                                              bufs=1)
                            if gate is not None:
                                nc.tensor.matmul(lg_ps[0:1, 0:1], gate, gate,
                                                 start=True, stop=True)
                            hT8s = ST[s]["hT8"][ts + 1]
                            for q0 in range(c0, c1, 512):
                                q1 = min(c1, q0 + 512)
                                for j in range(RCN // 2):
                                    hb = hT8s[:, j * 64:j * 64 + 32]
                                    lhsT = AP(hb.tensor, hb.offset,
                                              [hb.ap[0], [32, 2], [1, 32]])
                                    lg8 = logit8_s[j][:, 0:1]
                                    rhs = AP(lg8.tensor, lg8.offset + q0,
                                             [lg8.ap[0], [VP, 2],
                                              [1, q1 - q0]])
                                    nc.tensor.matmul(
                                        lg_ps[:, q0 - c0:q1 - c0], lhsT, rhs,
                                        perf_mode=(mybir.MatmulPerfMode
                                                   .DoubleRow),
                                        start=(j == 0),
                                        stop=(j == RCN // 2 - 1))
                            nc.vector.scalar_tensor_tensor(
                                ST[s]["lg"][ts][:, c0:c1],
                                lg_ps[:, 0:c1 - c0], 1.0 / LOGIT_SCALE,
                                logit_b_s[0:32, c0:c1],
                                op0=ALU.mult, op1=ALU.add)

                        def logit_stats(s, ts):
                            sl = (ts % 2) * 2
                            junk = work.tile([32, VP], BF16, tag=f"junk{s}",
                                             name="junk", bufs=1)
                            s_f = work.tile([32, 1], FP32, tag=f"s_f{s}",
                                            name="s_f", bufs=2)
                            nc.scalar.activation(junk[:], ST[s]["lg"][ts][:],
                                                 AF.Exp, accum_out=s_f[:])
                            nc.vector.tensor_copy(
                                statb[s][:, sl + 1:sl + 2], s_f[:])

                        def emit_logZ(s, statg_ap, nj, j0):
                            sview = statg_ap.rearrange("p (c j) -> p j c",
                                                       j=nj)
                            S_t = work.tile([32, 1], FP32, tag=f"S_t{s}",
                                            name="S_t", bufs=2)
                            nc.vector.tensor_reduce(
                                S_t[:], sview[:, j0 + 1:j0 + 2, :],
                                axis=AX.X, op=ALU.add)
                            lnS = work.tile([32, 1], FP32, tag=f"lnS{s}",
                                            name="lnS", bufs=2)
                            nc.scalar.activation(lnS[:], S_t[:], AF.Ln)
                            nZ = work.tile([32, 1], FP32, tag=f"nZ{s}",
                                           name="nZ", bufs=2)
                            nc.vector.tensor_scalar(nZ[:], lnS[:], -1.0,
                                                    None, op0=ALU.mult)
                            return nZ

                        def emit_out(s, ts):
                            lp_t = work.tile([32, VP], FP32, tag=f"lp{s}",
                                             name="lp_t", bufs=1)
                            nc.vector.tensor_scalar(
                                lp_t[:], ST[s]["lg"][ts][:],
                                ST[s]["nZ"][ts][:], None, op0=ALU.add)
                            nc.sync.dma_start(
                                out=out_d[ts * B + s * 32:
                                          ts * B + s * 32 + 32, :],
                                in_=lp_t[:])
                            del ST[s]["lg"][ts], ST[s]["nZ"][ts]

                        def sums_emit(s, t):
                            sums_ps = psum.tile([32, NGATE], FP32,
                                                tag="sums", name=f"sums{s}",
                                                bufs=1)
                            hTs = ST[s]["hT"][t]
                            for c0 in (0, 512):
                                c1 = min(NGATE, c0 + 512)
                                for kc in range(3):
                                    nc.tensor.matmul(
                                        sums_ps[:, c0:c1],
                                        xtT_s[kc][:, t * B + s * 32:
                                                   t * B + s * 32 + 32],
                                        i2hT_s[kc][:, c0:c1],
                                        start=(kc == 0),
                                        stop=(t == 0 and kc == 2))
                                if t >= 1:
                                    for rc in range(RCN):
                                        nc.tensor.matmul(
                                            sums_ps[:, c0:c1],
                                            hTs[:, rc * 32:(rc + 1) * 32],
                                            h2hT_s[rc][:, c0:c1],
                                            start=False,
                                            stop=(rc == RCN - 1))
                            sig3 = work.tile([32, 384], FP32, tag=f"sig3{s}",
                                             name="sig3", bufs=2)
                            nc.scalar.activation(sig3[:], sums_ps[:, 0:384],
                                                 AF.Tanh, scale=0.5)
                            sitr = work.tile([32, 256], FP32, tag=f"sitr{s}",
                                             name="sitr", bufs=2)
                            nc.vector.tensor_copy(sitr[:],
                                                  sums_ps[:, 384:640])
                            ST[s]["sig3"][t] = sig3
                            ST[s]["sitr"][t] = sitr

                        def attn_half(s, t):
                            agMx = work.tile([128, AGW], BF16,
                                             tag=f"agAx{s}", name="agAx",
                                             bufs=2)
                            if t >= 2:
                                nc.vector.tensor_copy(
                                    agMx[0:32, 64:66],
                                    statb[s][:, ((t - 2) % 2) * 2:
                                             ((t - 2) % 2) * 2 + 2])
                            else:
                                nc.vector.memset(agMx[:, 64:66], 0.0)
                            if t >= 1:
                                ah_ps = psum.tile([32, H], FP32, tag="mid",
                                                  name=f"ah_ps{s}", bufs=2)
                                hT8p = ST[s]["hT8"][t]
                                for j in range(RCN // 2):
                                    hb = hT8p[:, j * 64:j * 64 + 32]
                                    lhsT = AP(hb.tensor, hb.offset,
                                              [hb.ap[0], [32, 2], [1, 32]])
                                    h2 = h2att8_s[j][:, 0:1]
                                    rhs = AP(h2.tensor, h2.offset,
                                             [h2.ap[0], [H, 2], [1, H]])
                                    nc.tensor.matmul(
                                        ah_ps[:], lhsT, rhs,
                                        perf_mode=(mybir.MatmulPerfMode
                                                   .DoubleRow),
                                        start=(j == 0),
                                        stop=(j == RCN // 2 - 1))
                                ah_sb = work.tile([32, H], BF16,
                                                  tag=f"ah_sb{s}",
                                                  name="ah_sb", bufs=1)
                                nc.vector.tensor_scalar(
                                    ah_sb[:], ah_ps[:], 1.0 / H2ATT_SCALE,
                                    None, op0=ALU.mult)
                                ahT_ps = psum.tile([128, HCN * 4], FP32,
                                                   tag="small",
                                                   name=f"ahT_ps{s}", bufs=2)
                                for hc in range(HCN):
                                    nc.tensor.matmul(
                                        ahT_ps[:, hc * 4:(hc + 1) * 4],
                                        ah_sb[:, hc * 128:(hc + 1) * 128],
                                        bsel_s[s][:], start=True, stop=True)
                                ahT = work.tile([128, HCN * 4], BF16,
                                                tag=f"ahT{s}", name="ahT",
                                                bufs=1)
                                nc.vector.tensor_copy(ahT[:], ahT_ps[:])

                            e_ps = psum.tile([4, L], FP32, tag="small",
                                             name=f"e_ps{s}", bufs=2)
                            n0 = s * 4 * L
                            for hc in range(HCN):
                                if t >= 1:
                                    dp = work.tile([128, 4 * L], BF16,
                                                   tag=f"dp{s}", name="dp",
                                                   bufs=2)
                                    nc.vector.tensor_tensor(
                                        dp[:].rearrange(
                                            "p (b l) -> p b l", b=4),
                                        p_attT[hc][:, n0:n0 + 4 * L]
                                        .rearrange("p (b l) -> p b l", b=4),
                                        bcast_free(
                                            ahT[:, hc * 4:(hc + 1) * 4], L),
                                        op=ALU.add)
                                dt_ = work.tile([128, 4 * L], BF16,
                                                tag=f"dt{s}", name="dt",
                                                bufs=2)
                                if t == 0:
                                    nc.scalar.activation(
                                        dt_[:],
                                        p_attT[hc][:, n0:n0 + 4 * L],
                                        AF.Tanh)
                                else:
                                    nc.scalar.activation(dt_[:], dp[:],
                                                         AF.Tanh)
                                for b4 in range(4):
                                    nc.tensor.matmul(
                                        e_ps[:],
                                        alpha_s[hc][:, s * 16 + b4 * 4:
                                                    s * 16 + b4 * 4 + 4],
                                        dt_[:, b4 * L:(b4 + 1) * L],
                                        start=(hc == 0 and b4 == 0),
                                        stop=(hc == HCN - 1 and b4 == 3))

                            nbias = work.tile([4, 1], FP32, tag=f"nb{s}",
                                              name="nbias", bufs=1)
                            nc.vector.tensor_reduce(nbias[:], e_ps[:],
                                                    axis=AX.X, op=ALU.max,
                                                    negate=True)
                            ebias = work.tile([4, 1], FP32, tag=f"eb{s}",
                                              name="ebias", bufs=1)
                            nc.vector.tensor_scalar(ebias[:], nbias[:],
                                                    LN16, None, op0=ALU.add)
                            wsum = work.tile([4, 1], FP32, tag=f"ws{s}",
                                             name="wsum", bufs=1)
                            nc.scalar.activation(w_bf[s][:, 0:L], e_ps[:],
                                                 AF.Exp, bias=ebias[:],
                                                 accum_out=wsum[:])
                            wp = work.tile([4, LP], FP8, tag=f"wp{s}",
                                           name="wp", bufs=1)
                            nc.vector.tensor_copy(
                                out=wp[:].rearrange(
                                    "p (lp g2 gp) -> p g2 gp lp",
                                    lp=16, g2=NGP),
                                in_=w_bf[s][:].rearrange(
                                    "p (g2 gp lp) -> p g2 gp lp",
                                    g2=NGP, gp=2))
                            wdr = dpool.tile([4, LP], FP8, tag=f"wdr{s}",
                                             name="wdr")
                            nc.sync.dma_start(out=wdr[:], in_=wp[:])
                            _qs = [nc.sync, nc.scalar] * 2
                            for b4 in range(4):
                                sl = stat_all[s * 64 + b4:
                                              s * 64 + 64:4, 0:1]
                                out_ap = AP(sl.tensor, sl.offset + b4,
                                            [sl.ap[0], [16, 2 * NGP]])
                                _qs[b4].dma_start(
                                    out=out_ap,
                                    in_=wdr[b4:b4 + 1, :].rearrange(
                                        "o (lp gg) -> (o lp) gg", lp=16))
                            rinv = work.tile([4, 1], FP32, tag=f"ri{s}",
                                             name="rinv", bufs=1)
                            nc.vector.reciprocal(rinv[:], wsum[:])

                            ar_sb = work.tile([4, F], BF16, tag=f"ar_sb{s}",
                                              name="ar_sb", bufs=1)
                            for qc in range(4):
                                f0 = qc * 512
                                ar_ps = psum.tile([4, 512], FP32, tag="ar",
                                                  name=f"ar_ps{s}", bufs=1)
                                for j in range(NGP):
                                    st_ = stat_all[s * 64:(s + 1) * 64,
                                                   j * 32:j * 32 + 4]
                                    lhsT = AP(st_.tensor, st_.offset,
                                              [st_.ap[0], [16, 2], [1, 4]])
                                    at = att8_t[j][s * 64:(s + 1) * 64,
                                                   0:1]
                                    rhs = AP(at.tensor, at.offset + f0,
                                             [at.ap[0], [F, 2], [1, 512]])
                                    nc.tensor.matmul(
                                        ar_ps[:], lhsT, rhs,
                                        perf_mode=(mybir.MatmulPerfMode
                                                   .DoubleRow),
                                        start=(j == 0),
                                        stop=(j == NGP - 1))
                                if qc % 2 == 0:
                                    nc.vector.tensor_scalar(
                                        ar_sb[:, f0:f0 + 512], ar_ps[:],
                                        rinv[:], None, op0=ALU.mult)
                                else:
                                    nc.scalar.activation(
                                        ar_sb[:, f0:f0 + 512], ar_ps[:],
                                        AF.Copy, scale=rinv[:])

                            arTo_ps = psum.tile([128, 64], BF16, tag="mid",
                                                name=f"arTo{s}", bufs=2)
                            for fc in range(FCN):
                                nc.tensor.transpose(
                                    arTo_ps[:, fc * 4:(fc + 1) * 4],
                                    ar_sb[:, fc * 128:(fc + 1) * 128],
                                    ident_s[0:4, 0:4])
                            nc.vector.tensor_copy(agMx[:, 0:64], arTo_ps[:])
                            return agMx

                        def cell_half(s, t, prev_ag):
                            arTc = work.tile([128, FCN * 32], BF16,
                                             tag=f"arTc{s}", name="arTc",
                                             bufs=1)
                            _ag = prev_ag[:]
                            arT_src = AP(_ag.tensor, _ag.offset,
                                         [[AGW, 128], [128 * AGW, NC],
                                          [1, FCN * 4]])
                            nc.sync.dma_start(
                                out=arTc[:].rearrange("p (c fb) -> p c fb",
                                                      c=NC),
                                in_=arT_src)
                            arT = work.tile([128, FCN * 32], BF16,
                                            tag=f"arT{s}", name="arT",
                                            bufs=1)
                            nc.vector.tensor_copy(
                                arT[:].rearrange("p (fc c b) -> p fc c b",
                                                 fc=FCN, c=NC),
                                arTc[:].rearrange("p (c fc b) -> p fc c b",
                                                  c=NC, fc=FCN))
                            ctx_ps = psum.tile([32, 256], FP32, tag="mid",
                                               name=f"ctx{s}", bufs=2)
                            for fc in range(FCN):
                                nc.tensor.matmul(
                                    ctx_ps[:],
                                    arT[:, fc * 32:(fc + 1) * 32],
                                    a2cT_s[fc][:], start=(fc == 0),
                                    stop=(fc == FCN - 1))
                            sig3 = ST[s]["sig3"].pop(t)
                            sitr = ST[s]["sitr"].pop(t)
                            itr1 = work.tile([32, GC], FP32, tag=f"it1{s}",
                                             name="itr1", bufs=1)
                            nc.vector.tensor_tensor(itr1[:], sitr[:, 0:128],
                                                    ctx_ps[:, 0:128],
                                                    op=ALU.add)
                            itr2 = work.tile([32, GC], FP32, tag=f"it2{s}",
                                             name="itr2", bufs=1)
                            nc.vector.tensor_tensor(itr2[:],
                                                    sitr[:, 128:256],
                                                    ctx_ps[:, 128:256],
                                                    op=ALU.add)
                            g_t = work.tile([32, GC], FP32, tag=f"g{s}",
                                            name="g_t", bufs=1)
                            nc.vector.tensor_tensor(g_t[:], itr1[:],
                                                    itr2[:], op=ALU.max)
                            a_t = work.tile([32, GC], FP32, tag=f"a{s}",
                                            name="a_t", bufs=1)
                            nc.vector.scalar_tensor_tensor(
                                a_t[:], sig3[:, 128:256], 1.0, c_st[s][:],
                                op0=ALU.add, op1=ALU.mult)
                            b_t = work.tile([32, GC], FP32, tag=f"b{s}",
                                            name="b_t", bufs=1)
                            nc.vector.scalar_tensor_tensor(
                                b_t[:], sig3[:, 0:128], 1.0, g_t[:],
                                op0=ALU.add, op1=ALU.mult)
                            nc2_t = work.tile([32, GC], FP32, tag=f"nc{s}",
                                              name="nc2", bufs=1)
                            nc.vector.tensor_tensor(nc2_t[:], a_t[:],
                                                    b_t[:], op=ALU.add)
                            nc.vector.tensor_scalar(c_st[s][:], nc2_t[:],
                                                    0.5, None, op0=ALU.mult)
                            tnc = work.tile([32, GC], FP32, tag=f"tnc{s}",
                                            name="tnc", bufs=1)
                            nc.scalar.activation(tnc[:], nc2_t[:], AF.Tanh,
                                                 scale=0.5)
                            nh2 = work.tile([32, GC], BF16, tag=f"nh2{s}",
                                            name="nh2", bufs=1)
                            nc.vector.scalar_tensor_tensor(
                                nh2[:], sig3[:, 256:384], 1.0, tnc[:],
                                op0=ALU.add, op1=ALU.mult)
                            nhT_ps = psum.tile([GC, 32], BF16, tag="small",
                                               name=f"nhT{s}", bufs=2)
                            nc.tensor.transpose(nhT_ps[:], nh2[:],
                                                ident_s[0:32, 0:32])
                            nhT_sb = work.tile([GC, 32], BF16,
                                               tag=f"nhT_sb{s}",
                                               name="nhT_sb", bufs=2)
                            nc.vector.tensor_copy(nhT_sb[:], nhT_ps[:])
                            return nhT_sb

                        def read_hT(s, prev_ag):
                            hT_new = hpool.tile([128, RCN * 32], BF16,
                                                tag=f"hT{s}", name="hT_new")
                            nc.sync.dma_start(
                                out=hT_new[:].rearrange(
                                    "rl (rc b) -> rl rc b", rc=RCN),
                                in_=prev_ag[:].rearrange(
                                    "(rc rl) b -> rl rc b", rc=RCN))
                            ST[s]["hT"].append(hT_new)
                            h8 = hpool.tile([128, RCN * 32], FP8,
                                            tag=f"hT8{s}", name="hT8_new")
                            nc.gpsimd.tensor_copy(h8[:], hT_new[:])
                            ST[s]["hT8"].append(h8)

                        for t in range(t_steps):
                            for s in range(2):
                                # attention + AG_A
                                agAx = attn_half(s, t)
                                agA_in = dpool.tile([128, AGW], BF16,
                                                    tag=f"agA_in{s}",
                                                    name="agA_in")
                                nc.sync.dma_start(out=agA_in[:],
                                                  in_=agAx[:])
                                if no_cc:
                                    nc.sync.dma_start(
                                        out=agA_out[s][t][0:128, :],
                                        in_=agA_in[:])
                                else:
                                    nc.gpsimd.collective_compute(
                                        "AllGather", ALU.bypass,
                                        replica_groups=RG,
                                        ins=[agA_in.opt()],
                                        outs=[agA_out[s][t][:]])
                                # window 1
                                sums_emit(s, t)
                                if t >= 1:
                                    sp = t - 1
                                    ST[s]["lg"][sp] = work.tile(
                                        [32, VP], FP32, tag=f"lg_sb{s}",
                                        name=f"lg_sb{s}_{sp}", bufs=3)
                                    gsbA = work.tile([1, 1], BF16,
                                                     tag=f"gsbA{s}",
                                                     name="gsbA", bufs=2)
                                    nc.sync.dma_start(out=gsbA[:],
                                                      in_=agA_in[0:1, 0:1])
                                    logit_mms(s, sp, 0, 512)
                                    logit_mms(s, sp, 512, 1024,
                                              gate=gsbA[:])

                                # cell + AG_H
                                nhT_sb = cell_half(s, t, agA_out[s][t])
                                agH_in = dpool.tile([GC, 32], BF16,
                                                    tag=f"agH_in{s}",
                                                    name="agH_in")
                                nc.sync.dma_start(out=agH_in[:],
                                                  in_=nhT_sb[:])
                                if no_cc:
                                    nc.sync.dma_start(
                                        out=agH_out[s][t][0:GC, :],
                                        in_=agH_in[:])
                                else:
                                    nc.gpsimd.collective_compute(
                                        "AllGather", ALU.bypass,
                                        replica_groups=RG,
                                        ins=[agH_in.opt()],
                                        outs=[agH_out[s][t][:]])
                                # window 2
                                gsbH = work.tile([1, 1], BF16,
                                                 tag=f"gsbH{s}",
                                                 name="gsbH", bufs=2)
                                nc.sync.dma_start(out=gsbH[:],
                                                  in_=agH_in[0:1, 0:1])
                                if t >= 1:
                                    sp = t - 1
                                    logit_mms(s, sp, 1024, VP,
                                              gate=gsbH[:])
                                    logit_stats(s, sp)
                                if t >= 2:
                                    statg = work.tile([32, 2 * NC], BF16,
                                                      tag=f"statg{s}",
                                                      name="statg", bufs=2)
                                    _agp = agA_out[s][t][:]
                                    statg_src = AP(
                                        _agp.tensor, _agp.offset + 64,
                                        [[AGW, 32], [128 * AGW, NC],
                                         [1, 2]])
                                    nc.sync.dma_start(
                                        out=statg[:].rearrange(
                                            "p (c j) -> p c j", c=NC),
                                        in_=statg_src)
                                    ST[s]["nZ"][t - 2] = emit_logZ(
                                        s, statg[:], 2, 0)
                                    emit_out(s, t - 2)
                                    dumt = work.tile([1, 1], FP32,
                                                     tag=f"dumt{s}",
                                                     name="dumt", bufs=1)
                                    nc.scalar.activation(
                                        dumt[:], statb[s][0:1, 0:1],
                                        AF.Tanh)
                                read_hT(s, agH_out[s][t])

                        # ======== tail ========
                        for s in range(2):
                            sL = t_steps - 1
                            ST[s]["lg"][sL] = work.tile(
                                [32, VP], FP32, tag=f"lg_sb{s}",
                                name=f"lg_sb{s}_t", bufs=3)
                            logit_mms(s, sL, 0, 512)
                            logit_mms(s, sL, 512, 1024)
                            logit_mms(s, sL, 1024, VP)
                            logit_stats(s, sL)

                        agS_in = dpool.tile([64, 4], BF16, tag="agS_in",
                                            name="agS_in")
                        nc.sync.dma_start(out=agS_in[0:32, :],
                                          in_=statb[0][:])
                        nc.sync.dma_start(out=agS_in[32:64, :],
                                          in_=statb[1][:])
                        if no_cc:
                            nc.sync.dma_start(out=agS_out[0:64, :],
                                              in_=agS_in[:])
                        else:
                            nc.gpsimd.collective_compute(
                                "AllGather", ALU.bypass, replica_groups=RG,
                                ins=[agS_in.opt()], outs=[agS_out[:]])
                        for s in range(2):
                            statg2 = work.tile([32, 4 * NC], BF16,
                                               tag=f"stg2_{s}",
                                               name="statg2", bufs=1)
                            _ags = agS_out[:]
                            src = AP(_ags.tensor,
                                     _ags.offset + s * 32 * 4,
                                     [[4, 32], [64 * 4, NC], [1, 4]])
                            nc.sync.dma_start(out=statg2[:], in_=src)
                            for ts in (t_steps - 2, t_steps - 1):
                                ST[s]["nZ"][ts] = emit_logZ(
                                    s, statg2[:], 4, (ts % 2) * 2)
                                emit_out(s, ts)

                for rep in range(reps):
                    emit_rep(rep)

    nc.compile()
    return nc, sorted(probes)


_NC_CACHE = {}


def kernel(**inputs):
    """Full-input entry point: returns logp [B, T, V1] float32."""
    from concourse.bass_utils import run_bass_kernel_spmd
    in_maps = host_prep(inputs)
    if "nc" not in _NC_CACHE:
        _NC_CACHE["nc"], _ = build(T, (), reps=1)
    nc = _NC_CACHE["nc"]
    res = run_bass_kernel_spmd(nc, in_maps, list(range(NC)))
    outs = [res.results[c]["logp"] for c in range(NC)]
    full = np.concatenate(outs, axis=1)[:, :V1]          # [T*B, V1]
    # row order within a step is stream-major: b' = s*32 + c*4 + j maps to
    # global batch c*8 + s*4 + j
    gb = np.zeros(B, dtype=np.int64)
    for s in range(2):
        for c in range(NC):
            for j in range(4):
                gb[s * 32 + c * 4 + j] = c * 8 + s * 4 + j
    logp = np.zeros((B, T, V1), dtype=np.float32)
    logp[gb] = full.reshape(T, B, V1).transpose(1, 0, 2)
    return np.ascontiguousarray(logp)



# revision 51
# speedup vs baseline: 1.7258x; 1.7258x over previous
"""Attention-LSTM captioning model on 8 trn2 cores (8-way tensor parallel).

Gate/itr/vocab output dims sharded across cores (full B=64 per core);
attention batch-sharded (8 batches/core, selected via per-core one-hot bsel
data, since the SPMD program is identical on every core). Activations are
transposed [feature, batch]. Per step: AllGather(att_resT + piggybacked
log-softmax stats), AllGather(nh chunk). Sigmoid(x) is computed as
(tanh(x/2)+1)/2 so the step only needs the {tanh, exp} ACT table; the hidden
state is stored as 2*h with h-consuming weights pre-halved on the host.

Per-step schedule fills both AllGather stall windows: the logit matmul for
step t-1, the gates matmul for step t, per-step log-softmax stats (bf16-
rounded max trick so the stats collapse to 2 bf16 scalars per row riding the
next AG_A), and the final logZ-subtract + output DMA for step t-2.
"""
import numpy as np
import ml_dtypes

import concourse.bacc as bacc
import concourse.mybir as mybir
import concourse.tile as tile
from concourse.ap import AP
from concourse.bass_utils import run_bass_kernel_spmd

BF16_NP = ml_dtypes.bfloat16
FP32 = mybir.dt.float32
BF16 = mybir.dt.bfloat16
AF = mybir.ActivationFunctionType
ALU = mybir.AluOpType
AX = mybir.AxisListType

B, T, R, H, F, E, L, V1 = 64, 20, 1024, 512, 2048, 300, 196, 12001
NC = 8
BMY = B // NC
GC = R // NC              # 128
NGATE = 5 * GC            # 640
VP = 1504
LP = 224
NG = LP // 16             # 14
NGP = NG // 2             # 7 fp8 DoubleRow pairs
EP = 384
HCN = H // 128            # 4
FCN = F // 128            # 16
RCN = R // 128            # 8
NBL = BMY * L             # 1568
FP8 = mybir.dt.float8e4
F8_NP = ml_dtypes.float8_e4m3
LOGIT_SCALE = 32.0        # fp8 range prescale on logit weights
H2ATT_SCALE = 128.0       # fp8 range prescale on h2att weights
CTX_SCALE = 64.0          # fp8 range prescale on ctx2att weights
LN16 = 2.772588722239781  # attention weights scaled by 16 for fp8 range


def _bf(x):
    return np.ascontiguousarray(np.asarray(x, dtype=np.float32)).astype(BF16_NP)


def _f8(x):
    return np.ascontiguousarray(np.asarray(x, dtype=np.float32)).astype(F8_NP)


def bcast_free(ap, n):
    """Append a step-0 free dim of size n to an AP (broadcast)."""
    return AP(ap.tensor, ap.offset, list(ap.ap) + [[0, n]])


def host_prep(inputs):
    seq = np.asarray(inputs["seq"])
    att = np.asarray(inputs["att_feats"], dtype=np.float32)
    embed_w = np.asarray(inputs["embed_w"], dtype=np.float32)
    ctx2att_w = np.asarray(inputs["ctx2att_w"], dtype=np.float32)
    ctx2att_b = np.asarray(inputs["ctx2att_b"], dtype=np.float32)
    h2att_w = np.asarray(inputs["h2att_w"], dtype=np.float32)
    h2att_b = np.asarray(inputs["h2att_b"], dtype=np.float32)
    alpha_w = np.asarray(inputs["alpha_w"], dtype=np.float32)
    i2h_w = np.asarray(inputs["i2h_w"], dtype=np.float32)
    i2h_b = np.asarray(inputs["i2h_b"], dtype=np.float32)
    h2h_w = np.asarray(inputs["h2h_w"], dtype=np.float32)
    h2h_b = np.asarray(inputs["h2h_b"], dtype=np.float32)
    a2c_w = np.asarray(inputs["a2c_w"], dtype=np.float32)
    a2c_b = np.asarray(inputs["a2c_b"], dtype=np.float32)
    logit_w = np.asarray(inputs["logit_w"], dtype=np.float32)
    logit_b = np.asarray(inputs["logit_b"], dtype=np.float32)

    xt = embed_w[seq]                                    # [B, T, E]
    xtT = np.zeros((EP, T * B), dtype=np.float32)
    xtT[:E] = xt.transpose(2, 1, 0).reshape(E, T * B)
    xtT[E] = 1.0
    xtT = _bf(xtT)
    bias_gate = i2h_b + h2h_b

    in_maps = []
    for c in range(NC):
        m = {"xtT": xtT}
        grows = np.concatenate([np.arange(gg * R + c * GC, gg * R + (c + 1) * GC)
                                for gg in range(5)])
        i2hT = np.zeros((EP, NGATE), dtype=np.float32)
        i2hT[:E] = i2h_w[grows, :].T
        i2hT[E] = bias_gate[grows]
        # fold the a2c bias into the itr-part gate bias (itr = sums + ctx)
        arows_b = np.concatenate([np.arange(c * GC, (c + 1) * GC),
                                  np.arange(R + c * GC, R + (c + 1) * GC)])
        i2hT[E, 384:640] += a2c_b[arows_b]
        m["i2hT"] = _bf(i2hT)
        m["h2hT"] = _bf(h2h_w[grows, :].T * 0.5)
        h2a = (h2att_w.T * (0.5 * H2ATT_SCALE)).reshape(RCN, 128, H)
        m["h2att8"] = _f8(np.concatenate([h2a[0::2], h2a[1::2]],
                                         axis=2).reshape(RCN // 2 * 128,
                                                         2 * H))
        cx = (ctx2att_w.T * CTX_SCALE).reshape(FCN, 128, H)
        m["ctxT8"] = _f8(np.concatenate([cx[0::2], cx[1::2]],
                                        axis=2).reshape(FCN // 2 * 128,
                                                        2 * H))
        # h2att_bias folded in: dot = tanh(p_att + h@h2attT) with both
        # biases additive per-h element (pre-scaled: pa psum is CTX_SCALE x)
        m["ctx_bias"] = _bf((ctx2att_b + h2att_b)[None, :] * CTX_SCALE)
        amy = att[c * BMY:(c + 1) * BMY]                 # [8, L, F]
        at8 = amy.transpose(2, 0, 1).reshape(FCN, 128, NBL)
        m["attT8"] = _f8(np.concatenate([at8[0::2], at8[1::2]],
                                        axis=2).reshape(FCN // 2 * 128,
                                                        2 * NBL))
        alb = np.zeros((NG * 16, 8, F), dtype=np.float32)
        alb[:L] = amy.transpose(1, 0, 2)
        alb = alb.reshape(NG, 128, F)
        # fp8 DoubleRow pair packing: tile j holds groups (2j | 2j+1)
        m["att_lb8"] = _f8(np.concatenate([alb[0::2], alb[1::2]],
                                          axis=2).reshape(NGP * 128, 2 * F))
        ac = np.zeros((HCN * 128, 64), dtype=np.float32)
        for b in range(BMY):
            ac[:, b * 8 + b] = alpha_w[0]
        m["alpha_cols"] = _bf(ac)
        arows = np.concatenate([np.arange(c * GC, (c + 1) * GC),
                                np.arange(R + c * GC, R + (c + 1) * GC)])
        m["a2cT"] = _bf(a2c_w[arows, :].T)
        vrows = np.arange(c * VP, (c + 1) * VP)
        lw = np.zeros((R, VP), dtype=np.float32)
        lb = np.full((1, VP), -1e30, dtype=np.float32)
        valid = vrows < V1
        lw[:, valid] = logit_w[vrows[valid], :].T * 0.5
        lb[0, valid] = logit_b[vrows[valid]]
        lwr = (lw * LOGIT_SCALE).reshape(RCN, 128, VP)
        m["logit8"] = _f8(np.concatenate([lwr[0::2], lwr[1::2]],
                                         axis=2).reshape(RCN // 2 * 128,
                                                         2 * VP))
        m["logit_bias"] = lb
        m["ident"] = _bf(np.eye(128))
        bsel = np.zeros((B, BMY), dtype=np.float32)
        for j in range(BMY):
            bsel[c * BMY + j, j] = 1.0
        m["bsel"] = _bf(bsel)
        in_maps.append(m)
    return in_maps


def build(t_steps=T, probes=(), reps=1, no_cc=False):
    nc = bacc.Bacc("TRN2", target_bir_lowering=False, debug=False,
                   num_devices=NC)
    probes = set(probes)
    RG = [list(range(NC))]
    AGW = 130                 # agA payload: 128 arT cols + 2 stat cols

    def din(name, shape, dt=BF16):
        return nc.dram_tensor(name, shape, dt, kind="ExternalInput")

    xtT_d = din("xtT", [EP, T * B])
    i2hT_d = din("i2hT", [EP, NGATE])
    h2hT_d = din("h2hT", [R, NGATE])
    h2att8_d = din("h2att8", [RCN // 2 * 128, 2 * H], FP8)
    ctxT8_d = din("ctxT8", [FCN // 2 * 128, 2 * H], FP8)
    ctx_b_d = din("ctx_bias", [1, H])
    attT8_d = din("attT8", [FCN // 2 * 128, 2 * NBL], FP8)
    att_lb8_d = din("att_lb8", [NGP * 128, 2 * F], FP8)
    alpha_d = din("alpha_cols", [HCN * 128, 64])
    a2cT_d = din("a2cT", [F, 256])
    logit8_d = din("logit8", [RCN // 2 * 128, 2 * VP], FP8)
    logit_b_d = din("logit_bias", [1, VP], FP32)
    ident_d = din("ident", [128, 128])
    bsel_d = din("bsel", [B, BMY])

    out_d = nc.dram_tensor("logp", [t_steps * B, VP], FP32,
                           kind="ExternalOutput")
    agA_out_r = [[nc.dram_tensor(f"agA_out_{rp}_{t}", [NC * 128, AGW], BF16,
                                 addr_space="Shared") for t in range(t_steps)]
                 for rp in range(reps)]
    agH_out_r = [[nc.dram_tensor(f"agH_out_{rp}_{t}", [R, B], BF16,
                                 addr_space="Shared") for t in range(t_steps)]
                 for rp in range(reps)]
    agS_out_r = [nc.dram_tensor(f"agS_out_{rp}", [NC * 64, 4], BF16,
                                addr_space="Shared") for rp in range(reps)]

    with tile.TileContext(nc) as tc:
        with (
            tc.tile_pool(name="wpool", bufs=1) as wpool,
            tc.tile_pool(name="hpool", bufs=4) as hpool,
            tc.tile_pool(name="psum", bufs=1, space="PSUM") as psum,
            tc.tile_pool(name="dram", bufs=4, space="DRAM") as dpool,
        ):
            def probe_(name, src_ap, shape, dt):
                pd = nc.dram_tensor(f"probe_{name}", list(shape), dt,
                                    kind="ExternalOutput")
                nc.sync.dma_start(out=pd[:], in_=src_ap)

            def load_chunks(pool, dram, cols, n, tag, dt=BF16):
                ts = []
                for i in range(n):
                    t_ = pool.tile([128, cols], dt, tag=f"{tag}{i}",
                                   name=f"{tag}{i}")
                    nc.sync.dma_start(out=t_[:],
                                      in_=dram[i * 128:(i + 1) * 128, :])
                    ts.append(t_)
                return ts

            logit8_s = load_chunks(wpool, logit8_d, 2 * VP, RCN // 2,
                                   "logit8", dt=FP8)
            logit_b_s = wpool.tile([64, VP], FP32, tag="logitb",
                                   name="logitb")
            _lb_src = AP(logit_b_d[:].tensor, logit_b_d[:].offset,
                         [[0, 64], [1, VP]])
            nc.sync.dma_start(out=logit_b_s[:], in_=_lb_src)
            ident_s = wpool.tile([128, 128], BF16, tag="ident", name="ident")
            nc.sync.dma_start(out=ident_s[:], in_=ident_d[:])
            ones64 = wpool.tile([1, B], BF16, tag="ones64", name="ones64")
            nc.vector.memset(ones64[:], 1.0)

            with tc.tile_pool(name="w1pool", bufs=1) as w1pool:
                xtT_s = load_chunks(w1pool, xtT_d, T * B, 3, "xtT")
                i2hT_s = load_chunks(w1pool, i2hT_d, NGATE, 3, "i2hT")
                h2hT_s = load_chunks(w1pool, h2hT_d, NGATE, RCN, "h2hT")
                h2att8_s = load_chunks(w1pool, h2att8_d, 2 * H, RCN // 2,
                                       "h2att8", dt=FP8)
                att_lb8_s = load_chunks(w1pool, att_lb8_d, 2 * F, NGP,
                                        "attlb8", dt=FP8)
                alpha_s = load_chunks(w1pool, alpha_d, 64, HCN, "alpha")
                a2cT_s = load_chunks(w1pool, a2cT_d, 256, FCN, "a2cT")
                bsel_s = w1pool.tile([B, BMY], BF16, tag="bsel", name="bsel")
                nc.sync.dma_start(out=bsel_s[:], in_=bsel_d[:])
                ctx_b_s = w1pool.tile([1, H], BF16, tag="ctxb", name="ctxb")
                nc.sync.dma_start(out=ctx_b_s[:], in_=ctx_b_d[:])
                onesNBL = w1pool.tile([1, NBL], BF16, tag="onesNBL",
                                      name="onesNBL")
                nc.vector.memset(onesNBL[:], 1.0)
                p_attT = [w1pool.tile([128, NBL], BF16, tag=f"pattT{hc}",
                                      name=f"pattT{hc}")
                          for hc in range(HCN)]
                # col = g2*32 + gp*16 + b: DoubleRow pair (gp) stride 16
                # (needs step%16==0) and the scatter dest merges to 2 dims
                stat_all = w1pool.tile([128, LP], FP8, tag="stat_all",
                                       name="stat_all")
                nc.vector.memset(stat_all[:], 0.0)
                w_bf = w1pool.tile([BMY, LP], FP8, tag="w_bf", name="w_bf")
                nc.vector.memset(w_bf[:], 0.0)
                c_st = w1pool.tile([B, GC], FP32, tag="c_st", name="c_st")
                statb = w1pool.tile([64, 4], BF16, tag="statb", name="statb")

                def emit_rep(rep):
                    agA_out = agA_out_r[rep]
                    agH_out = agH_out_r[rep]
                    agS_out = agS_out_r[rep]

                    def probe(name, src_ap, shape, dt):
                        if rep == 0 and name in probes:
                            probe_(name, src_ap, shape, dt)

                    nc.vector.memset(c_st[:], 0.0)
                    nc.vector.memset(statb[:], 0.0)
                    hT = hpool.tile([128, RCN * 64], BF16, tag="hT",
                                    name="hT0")
                    nc.vector.memset(hT[:], 0.0)
                    hT_hist = [hT]
                    hT8 = hpool.tile([128, RCN * 64], FP8, tag="hT8",
                                     name="hT80")
                    nc.vector.memset(hT8[:], 0.0)
                    hT8_hist = [hT8]

                    # ---------- phase 0 (fp8 DoubleRow over fc pairs) ------
                    with (
                        tc.tile_pool(name=f"ctxpool{rep}", bufs=1) as ctxpool,
                        tc.tile_pool(name=f"stream{rep}", bufs=3) as stream,
                    ):
                        ctxT8_s = load_chunks(ctxpool, ctxT8_d, 2 * H,
                                              FCN // 2, "ctxT8", dt=FP8)
                        QW = 392
                        for q in range(4):
                            n0 = q * QW
                            _pa_tags = ["sums", "mid", "ar", "small"]
                            pa_ps = [psum.tile([128, QW], FP32,
                                               tag=_pa_tags[hc],
                                               name=f"pa{hc}", bufs=1)
                                     for hc in range(HCN)]
                            for jf in range(FCN // 2):
                                at = stream.tile([128, 2 * QW], FP8,
                                                 tag="attTq", name="attTq")
                                a8 = attT8_d[jf * 128:(jf + 1) * 128, 0:1]
                                nc.sync.dma_start(
                                    out=at[:],
                                    in_=AP(a8.tensor, a8.offset + n0,
                                           [a8.ap[0], [NBL, 2], [1, QW]]))
                                for hc in range(HCN):
                                    cx = ctxT8_s[jf][:, 0:1]
                                    lhsT = AP(cx.tensor,
                                              cx.offset + hc * 128,
                                              [cx.ap[0], [H, 2], [1, 128]])
                                    rhs = at[:].rearrange(
                                        "p (two q) -> p two q", two=2)
                                    nc.tensor.matmul(
                                        pa_ps[hc][:], lhsT, rhs,
                                        perf_mode=(mybir.MatmulPerfMode
                                                   .DoubleRow),
                                        start=(jf == 0), stop=False)
                            for hc in range(HCN):
                                nc.tensor.matmul(
                                    pa_ps[hc][:],
                                    ctx_b_s[:, hc * 128:(hc + 1) * 128],
                                    onesNBL[:, n0:n0 + QW], start=False,
                                    stop=True)
                                nc.vector.tensor_scalar(
                                    p_attT[hc][:, n0:n0 + QW], pa_ps[hc][:],
                                    1.0 / CTX_SCALE, None, op0=ALU.mult)
                    probe("p_attT0", p_attT[0][:], [128, NBL], BF16)

                    # ---------- phase 1 ----------
                    with tc.tile_pool(name=f"work1_{rep}", bufs=1) as work:
                        lg_sbs = {}      # block s -> lg_sb tile
                        nlogZs = {}      # block s -> nlogZ tile

                        def logit_mms(s, c0, c1, gate=None):
                            """Emit logit matmuls for step s, col chunk.
                            `gate` (a tiny SBUF tile DMA'd from the AG
                            staging buffer) delays the matmuls until the
                            collective is in flight, so the scheduler can't
                            hoist them out of the AG stall window."""
                            lg_ps = psum.tile([64, 512], FP32, tag="lg",
                                              name=f"lg_ps_{s}_{c0}", bufs=2)
                            if gate is not None:
                                nc.tensor.matmul(lg_ps[0:1, 0:1],
                                                 gate, gate,
                                                 start=True, stop=True)
                            hT8s = hT8_hist[s + 1]
                            for q0 in range(c0, c1, 512):
                                q1 = min(c1, q0 + 512)
                                for j in range(RCN // 2):
                                    hb = hT8s[:, j * 128:j * 128 + 64]
                                    lhsT = AP(hb.tensor, hb.offset,
                                              [hb.ap[0], [64, 2], [1, 64]])
                                    lg8 = logit8_s[j][:, 0:1]
                                    rhs = AP(lg8.tensor, lg8.offset + q0,
                                             [lg8.ap[0], [VP, 2],
                                              [1, q1 - q0]])
                                    nc.tensor.matmul(
                                        lg_ps[:, q0 - c0:q1 - c0],
                                        lhsT, rhs,
                                        perf_mode=(mybir.MatmulPerfMode
                                                   .DoubleRow),
                                        start=(j == 0),
                                        stop=(j == RCN // 2 - 1))
                            lg_sb = lg_sbs[s]
                            nc.vector.scalar_tensor_tensor(
                                lg_sb[:, c0:c1], lg_ps[:, 0:c1 - c0],
                                1.0 / LOGIT_SCALE, logit_b_s[:, c0:c1],
                                op0=ALU.mult, op1=ALU.add)

                        def logit_stats(s):
                            """expsum for block s (|logits| <= ~51, so no
                            max-shift needed before exp); bf16 stat into
                            statb slot (s % 2)."""
                            lg_sb = lg_sbs[s]
                            sl = (s % 2) * 2
                            junk = work.tile([64, VP], BF16, tag="junk",
                                             name="junk", bufs=2)
                            s_f = work.tile([64, 1], FP32, tag="s_f",
                                            name="s_f", bufs=2)
                            nc.scalar.activation(junk[:], lg_sb[:], AF.Exp,
                                                 accum_out=s_f[:])
                            nc.vector.tensor_copy(statb[:, sl + 1:sl + 2],
                                                  s_f[:])

                        def emit_logZ(statg_ap, nj, j0):
                            """Combine gathered per-core expsums -> nlogZ
                            ([64, 1] f32, = -logZ)."""
                            sview = statg_ap.rearrange("p (c j) -> p j c",
                                                       j=nj)
                            S_t = work.tile([64, 1], FP32, tag="S_t",
                                            name="S_t", bufs=2)
                            nc.vector.tensor_reduce(
                                S_t[:], sview[:, j0 + 1:j0 + 2, :],
                                axis=AX.X, op=ALU.add)
                            lnS = work.tile([64, 1], FP32, tag="lnS",
                                            name="lnS", bufs=2)
                            nc.scalar.activation(lnS[:], S_t[:], AF.Ln)
                            nlogZ = work.tile([64, 1], FP32, tag="nlogZ",
                                              name="nlogZ", bufs=2)
                            nc.vector.tensor_scalar(nlogZ[:], lnS[:], -1.0,
                                                    None, op0=ALU.mult)
                            return nlogZ

                        def emit_out(s):
                            """Subtract logZ for block s and DMA out."""
                            lp_t = work.tile([64, VP], FP32, tag="lp_t",
                                             name="lp_t", bufs=2)
                            nc.vector.tensor_scalar(
                                lp_t[:], lg_sbs[s][:], nlogZs[s][:], None,
                                op0=ALU.add)
                            nc.sync.dma_start(
                                out=out_d[s * B:(s + 1) * B, :], in_=lp_t[:])
                            del lg_sbs[s], nlogZs[s]

                        for t in range(t_steps):
                            # ======== segment A: attention ========
                            agAx = work.tile([128, AGW], BF16, tag="agAx",
                                             name="agAx", bufs=2)
                            if t >= 2:
                                nc.vector.tensor_copy(
                                    agAx[0:64, 128:130],
                                    statb[:, ((t - 2) % 2) * 2:
                                          ((t - 2) % 2) * 2 + 2])
                            else:
                                nc.vector.memset(agAx[:, 128:130], 0.0)

                            if t >= 1:
                                ah_ps = psum.tile([B, H], FP32, tag="mid",
                                                  name="ah_ps", bufs=1)
                                hT8p = hT8_hist[t]
                                for j in range(RCN // 2):
                                    hb = hT8p[:, j * 128:j * 128 + 64]
                                    lhsT = AP(hb.tensor, hb.offset,
                                              [hb.ap[0], [64, 2], [1, 64]])
                                    h2 = h2att8_s[j][:, 0:1]
                                    rhs = AP(h2.tensor, h2.offset,
                                             [h2.ap[0], [H, 2], [1, H]])
                                    nc.tensor.matmul(
                                        ah_ps[:], lhsT, rhs,
                                        perf_mode=(mybir.MatmulPerfMode
                                                   .DoubleRow),
                                        start=(j == 0),
                                        stop=(j == RCN // 2 - 1))
                                ah_sb = work.tile([B, H], BF16, tag="ah_sb",
                                                  name="ah_sb", bufs=1)
                                nc.vector.tensor_scalar(
                                    ah_sb[:], ah_ps[:], 1.0 / H2ATT_SCALE,
                                    None, op0=ALU.mult)
                                ahT_ps = psum.tile([128, HCN * 8], FP32,
                                                   tag="small",
                                                   name="ahT_ps", bufs=1)
                                for hc in range(HCN):
                                    nc.tensor.matmul(
                                        ahT_ps[:, hc * 8:(hc + 1) * 8],
                                        ah_sb[:, hc * 128:(hc + 1) * 128],
                                        bsel_s[:], start=True, stop=True)
                                ahT = work.tile([128, HCN * 8], BF16,
                                                tag="ahT_sb", name="ahT_sb",
                                                bufs=1)
                                nc.vector.tensor_copy(ahT[:], ahT_ps[:])

                            e_ps = psum.tile([BMY, L], FP32, tag="small",
                                             name="e_ps", bufs=1)
                            HB = BMY // 2
                            for hc in range(HCN):
                                if t >= 1:
                                    dp = work.tile([128, NBL], BF16,
                                                   tag="dp", name="dp",
                                                   bufs=2)
                                dt_ = work.tile([128, NBL], BF16, tag="dt",
                                                name="dt", bufs=2)
                                for bh in range(2):
                                    c0, c1 = bh * HB * L, (bh + 1) * HB * L
                                    if t == 0:
                                        # h0 == 0 -> dot = tanh(p_att)
                                        nc.scalar.activation(
                                            dt_[:, c0:c1],
                                            p_attT[hc][:, c0:c1], AF.Tanh)
                                    else:
                                        nc.vector.tensor_tensor(
                                            dp[:, c0:c1].rearrange(
                                                "p (b l) -> p b l", b=HB),
                                            p_attT[hc][:, c0:c1].rearrange(
                                                "p (b l) -> p b l", b=HB),
                                            bcast_free(
                                                ahT[:, hc * 8 + bh * HB:
                                                    hc * 8 + (bh + 1) * HB],
                                                L),
                                            op=ALU.add)
                                        nc.scalar.activation(dt_[:, c0:c1],
                                                             dp[:, c0:c1],
                                                             AF.Tanh)
                                    for b in range(bh * HB, (bh + 1) * HB):
                                        nc.tensor.matmul(
                                            e_ps[:],
                                            alpha_s[hc][:,
                                                        b * 8:(b + 1) * 8],
                                            dt_[:, b * L:(b + 1) * L],
                                            start=(hc == 0 and b == 0),
                                            stop=(hc == HCN - 1 and
                                                  b == BMY - 1))

                            # fp8 range: w = exp(e - max_e + ln16) in (0,16].
                            # The exp's accum_out gives the pre-quantize sum;
                            # 1/sum folds into the psum copies below (the x16
                            # cancels in the normalization).
                            nbias = work.tile([BMY, 1], FP32, tag="nbias",
                                              name="nbias", bufs=1)
                            nc.vector.tensor_reduce(nbias[:], e_ps[:],
                                                    axis=AX.X, op=ALU.max,
                                                    negate=True)
                            ebias = work.tile([BMY, 1], FP32, tag="ebias",
                                              name="ebias", bufs=1)
                            nc.vector.tensor_scalar(ebias[:], nbias[:],
                                                    LN16, None, op0=ALU.add)
                            wsum = work.tile([BMY, 1], FP32, tag="wsum",
                                             name="wsum", bufs=1)
                            nc.scalar.activation(w_bf[:, 0:L], e_ps[:],
                                                 AF.Exp, bias=ebias[:],
                                                 accum_out=wsum[:])

                            # scatter unnormalized weights into the
                            # (lp, b)-diagonal layout via a DRAM round trip
                            # (reads split across the SP and ACT dma queues)
                            wp = work.tile([BMY, LP], FP8, tag="wp",
                                           name="wp", bufs=1)
                            nc.vector.tensor_copy(
                                out=wp[:].rearrange(
                                    "p (lp g2 gp) -> p g2 gp lp",
                                    lp=16, g2=NGP),
                                in_=w_bf[:].rearrange(
                                    "p (g2 gp lp) -> p g2 gp lp",
                                    g2=NGP, gp=2))
                            wdr = dpool.tile([BMY, LP], FP8, tag="wdr",
                                             name="wdr")
                            nc.sync.dma_start(out=wdr[:], in_=wp[:])
                            _qs = [nc.sync, nc.scalar] * 4
                            for b in range(BMY):
                                sl = stat_all[b:128:8, 0:1]
                                out_ap = AP(sl.tensor, sl.offset + b,
                                            [sl.ap[0], [16, 2 * NGP]])
                                _qs[b].dma_start(
                                    out=out_ap,
                                    in_=wdr[b:b + 1, :].rearrange(
                                        "o (lp gg) -> (o lp) gg", lp=16))

                            rinv = work.tile([BMY, 1], FP32, tag="rinv",
                                             name="rinv", bufs=1)
                            nc.vector.reciprocal(rinv[:], wsum[:])

                            ar_sb = work.tile([BMY, F], BF16, tag="ar_sb",
                                              name="ar_sb", bufs=1)
                            for half in range(2):
                                f0 = half * 1024
                                ar_ps = psum.tile([BMY, 1024], FP32,
                                                  tag="ar", name="ar_ps",
                                                  bufs=1)
                                for qf in range(2):
                                    for j in range(NGP):
                                        st = stat_all[:, j * 32:
                                                      j * 32 + 8]
                                        lhsT = AP(st.tensor, st.offset,
                                                  [st.ap[0], [16, 2],
                                                   [1, 8]])
                                        at = att_lb8_s[j][:, 0:1]
                                        rhs = AP(at.tensor,
                                                 at.offset + f0 + qf * 512,
                                                 [at.ap[0], [F, 2],
                                                  [1, 512]])
                                        nc.tensor.matmul(
                                            ar_ps[:, qf * 512:
                                                  (qf + 1) * 512],
                                            lhsT, rhs,
                                            perf_mode=(mybir.MatmulPerfMode
                                                       .DoubleRow),
                                            start=(j == 0),
                                            stop=(j == NGP - 1))
                                if half == 0:
                                    nc.vector.tensor_scalar(
                                        ar_sb[:, f0:f0 + 1024], ar_ps[:],
                                        rinv[:], None, op0=ALU.mult)
                                else:
                                    nc.scalar.activation(
                                        ar_sb[:, f0:f0 + 1024], ar_ps[:],
                                        AF.Copy, scale=rinv[:])

                            # transpose own att_res before the AllGather;
                            # stage straight from PSUM + statb to DRAM
                            arTo_ps = psum.tile([128, 128], BF16, tag="mid",
                                                name="arTo_ps", bufs=1)
                            for fc in range(FCN):
                                nc.tensor.transpose(
                                    arTo_ps[:, fc * 8:(fc + 1) * 8],
                                    ar_sb[:, fc * 128:(fc + 1) * 128],
                                    ident_s[0:BMY, 0:BMY])
                            nc.vector.tensor_copy(agAx[:, 0:128],
                                                  arTo_ps[:])
                            agA_in = dpool.tile([128, AGW], BF16,
                                                tag="agA_in", name="agA_in")
                            nc.sync.dma_start(out=agA_in[:], in_=agAx[:])
                            if no_cc:
                                nc.sync.dma_start(out=agA_out[t][0:128, :],
                                                  in_=agA_in[:])
                            else:
                                nc.gpsimd.collective_compute(
                                    "AllGather", ALU.bypass,
                                    replica_groups=RG,
                                    ins=[agA_in.opt()], outs=[agA_out[t][:]])

                            # ======== window 1 (during AG_A) ========
                            sums_ps = psum.tile([B, NGATE], FP32, tag="sums",
                                                name="sums", bufs=1)
                            for c0 in (0, 512):
                                c1 = min(NGATE, c0 + 512)
                                for kc in range(3):
                                    nc.tensor.matmul(
                                        sums_ps[:, c0:c1],
                                        xtT_s[kc][:, t * B:(t + 1) * B],
                                        i2hT_s[kc][:, c0:c1],
                                        start=(kc == 0),
                                        stop=(t == 0 and kc == 2))
                                if t >= 1:
                                    for rc in range(RCN):
                                        nc.tensor.matmul(
                                            sums_ps[:, c0:c1],
                                            hT[:, rc * 64:(rc + 1) * 64],
                                            h2hT_s[rc][:, c0:c1],
                                            start=False,
                                            stop=(rc == RCN - 1))
                            sig3 = work.tile([B, 384], FP32, tag="sig3",
                                             name="sig3", bufs=1)
                            nc.scalar.activation(sig3[:], sums_ps[:, 0:384],
                                                 AF.Tanh, scale=0.5)
                            sitr = work.tile([B, 256], FP32, tag="sitr",
                                             name="sitr", bufs=1)
                            nc.vector.tensor_copy(sitr[:],
                                                  sums_ps[:, 384:640])

                            if t >= 1:
                                s = t - 1
                                lg_sbs[s] = work.tile([64, VP], FP32,
                                                      tag="lg_sb",
                                                      name=f"lg_sb{s}",
                                                      bufs=3)
                                gsbA = work.tile([1, 1], BF16, tag="gsbA",
                                                 name="gsbA", bufs=2)
                                nc.sync.dma_start(out=gsbA[:],
                                                  in_=agA_in[0:1, 0:1])
                                logit_mms(s, 0, 512)
                                logit_mms(s, 512, 1024, gate=gsbA[:])

                            # ======== post-AG_A ========
                            # arTc cols: c*128 + fc*8 + b (one DMA),
                            # then DVE repack to fc-major for the matmul
                            arTc = work.tile([128, FCN * 64], BF16,
                                             tag="arTc", name="arTc",
                                             bufs=1)
                            _ag = agA_out[t][:]
                            arT_src = AP(_ag.tensor, _ag.offset,
                                         [[AGW, 128], [128 * AGW, NC],
                                          [1, FCN * BMY]])
                            nc.sync.dma_start(
                                out=arTc[:].rearrange("p (c fb) -> p c fb",
                                                      c=NC),
                                in_=arT_src)
                            arT = work.tile([128, FCN * 64], BF16, tag="arT",
                                            name="arT", bufs=1)
                            nc.vector.tensor_copy(
                                arT[:].rearrange("p (fc c b) -> p fc c b",
                                                 fc=FCN, c=NC),
                                arTc[:].rearrange("p (c fc b) -> p fc c b",
                                                  c=NC, fc=FCN))

                            ctx_ps = psum.tile([B, 256], FP32, tag="mid",
                                               name="ctx_ps", bufs=1)
                            for fc in range(FCN):
                                nc.tensor.matmul(
                                    ctx_ps[:], arT[:, fc * 64:(fc + 1) * 64],
                                    a2cT_s[fc][:], start=(fc == 0),
                                    stop=(fc == FCN - 1))

                            itr1 = work.tile([B, GC], FP32, tag="itr1",
                                             name="itr1", bufs=1)
                            nc.vector.tensor_tensor(itr1[:], sitr[:, 0:128],
                                                    ctx_ps[:, 0:128],
                                                    op=ALU.add)
                            itr2 = work.tile([B, GC], FP32, tag="itr2",
                                             name="itr2", bufs=1)
                            nc.vector.tensor_tensor(itr2[:],
                                                    sitr[:, 128:256],
                                                    ctx_ps[:, 128:256],
                                                    op=ALU.add)
                            g_t = work.tile([B, GC], FP32, tag="g_t",
                                            name="g_t", bufs=1)
                            nc.vector.tensor_tensor(g_t[:], itr1[:],
                                                    itr2[:], op=ALU.max)
                            a_t = work.tile([B, GC], FP32, tag="a_t",
                                            name="a_t", bufs=1)
                            nc.vector.scalar_tensor_tensor(
                                a_t[:], sig3[:, 128:256], 1.0, c_st[:],
                                op0=ALU.add, op1=ALU.mult)
                            b_t = work.tile([B, GC], FP32, tag="b_t",
                                            name="b_t", bufs=1)
                            nc.vector.scalar_tensor_tensor(
                                b_t[:], sig3[:, 0:128], 1.0, g_t[:],
                                op0=ALU.add, op1=ALU.mult)
                            nc2_t = work.tile([B, GC], FP32, tag="nc2",
                                              name="nc2", bufs=1)
                            nc.vector.tensor_tensor(nc2_t[:], a_t[:],
                                                    b_t[:], op=ALU.add)
                            nc.vector.tensor_scalar(c_st[:], nc2_t[:], 0.5,
                                                    None, op0=ALU.mult)
                            tnc = work.tile([B, GC], FP32, tag="tnc",
                                            name="tnc", bufs=1)
                            nc.scalar.activation(tnc[:], nc2_t[:], AF.Tanh,
                                                 scale=0.5)
                            nh2 = work.tile([B, GC], BF16, tag="nh2",
                                            name="nh2", bufs=1)
                            nc.vector.scalar_tensor_tensor(
                                nh2[:], sig3[:, 256:384], 1.0, tnc[:],
                                op0=ALU.add, op1=ALU.mult)

                            nhT_ps = psum.tile([GC, B], BF16, tag="small",
                                               name="nhT_ps", bufs=1)
                            nc.tensor.transpose(nhT_ps[:], nh2[:],
                                                ident_s[0:B, 0:B])
                            nhT_sb = work.tile([GC, B], BF16, tag="nhT_sb",
                                               name="nhT_sb", bufs=1)
                            nc.vector.tensor_copy(nhT_sb[:], nhT_ps[:])
                            agH_in = dpool.tile([GC, B], BF16, tag="agH_in",
                                                name="agH_in")
                            nc.sync.dma_start(out=agH_in[:], in_=nhT_sb[:])
                            if no_cc:
                                nc.sync.dma_start(out=agH_out[t][0:GC, :],
                                                  in_=agH_in[:])
                            else:
                                nc.gpsimd.collective_compute(
                                    "AllGather", ALU.bypass,
                                    replica_groups=RG,
                                    ins=[agH_in.opt()], outs=[agH_out[t][:]])

                            # ======== window 2 (during AG_H) ========
                            gH = agH_in[0:1, 0:1]
                            gsbH = work.tile([1, 1], BF16, tag="gsbH",
                                             name="gsbH", bufs=2)
                            nc.sync.dma_start(out=gsbH[:], in_=gH)
                            if t >= 1:
                                s = t - 1
                                logit_mms(s, 1024, VP, gate=gsbH[:])
                                logit_stats(s)
                            if t >= 2:
                                statg = work.tile([64, 2 * NC], BF16,
                                                  tag="statg", name="statg",
                                                  bufs=2)
                                nc.sync.dma_start(out=statg[0:1, 0:1],
                                                  in_=gH)
                                statg_src = AP(_ag.tensor, _ag.offset + 128,
                                               [[AGW, 64], [128 * AGW, NC],
                                                [1, 2]])
                                nc.sync.dma_start(
                                    out=statg[:].rearrange(
                                        "p (c j) -> p c j", c=NC),
                                    in_=statg_src)
                                nlogZs[t - 2] = emit_logZ(statg[:], 2, 0)
                                emit_out(t - 2)
                                # prefetch the tanh act-table set while the
                                # AG still runs (Ln swapped the set out)
                                dumt = work.tile([1, 1], FP32, tag="dumt",
                                                 name="dumt", bufs=1)
                                nc.scalar.activation(dumt[:],
                                                     statb[0:1, 0:1],
                                                     AF.Tanh)

                            # ======== post-AG_H ========
                            hT_new = hpool.tile([128, RCN * 64], BF16,
                                                tag="hT", name="hT_new")
                            nc.sync.dma_start(
                                out=hT_new[:].rearrange(
                                    "rl (rc b) -> rl rc b", rc=RCN),
                                in_=agH_out[t][:].rearrange(
                                    "(rc rl) b -> rl rc b", rc=RCN))
                            hT_hist.append(hT_new)
                            hT = hT_new
                            hT8_new = hpool.tile([128, RCN * 64], FP8,
                                                 tag="hT8", name="hT8_new")
                            nc.gpsimd.tensor_copy(hT8_new[:], hT_new[:])
                            hT8_hist.append(hT8_new)

                            if t == 0:
                                probe("w0", w_bf[:], [BMY, LP], BF16)
                                probe("statall0", stat_all[:],
                                      [128, NG * 8], BF16)
                                probe("ar0", ar_sb[:], [BMY, F], BF16)
                                probe("nh20", nh2[:], [B, GC], BF16)
                                probe("hT1", hT_new[:], [128, RCN * 64],
                                      BF16)
                            if t == 1:
                                probe("lg0", lg_sbs[0][:], [64, VP], FP32)
                                probe("arT1", arT[:], [128, FCN * 64], BF16)

                        # ======== tail ========
                        s = t_steps - 1
                        lg_sbs[s] = work.tile([64, VP], FP32, tag="lg_sb",
                                              name=f"lg_sb{s}", bufs=3)
                        logit_mms(s, 0, 512)
                        logit_mms(s, 512, 1024)
                        logit_mms(s, 1024, VP)
                        logit_stats(s)

                        agS_in = dpool.tile([64, 4], BF16, tag="agS_in",
                                            name="agS_in")
                        nc.sync.dma_start(out=agS_in[:], in_=statb[:])
                        if no_cc:
                            nc.sync.dma_start(out=agS_out[0:64, :],
                                              in_=agS_in[:])
                        else:
                            nc.gpsimd.collective_compute(
                                "AllGather", ALU.bypass, replica_groups=RG,
                                ins=[agS_in.opt()], outs=[agS_out[:]])
                        statg2 = work.tile([64, 4 * NC], BF16, tag="statg2",
                                           name="statg2", bufs=1)
                        _ags = agS_out[:]
                        statg2_src = AP(_ags.tensor, _ags.offset,
                                        [[4, 64], [64 * 4, NC], [1, 4]])
                        nc.sync.dma_start(out=statg2[:], in_=statg2_src)
                        for s in (t_steps - 2, t_steps - 1):
                            nlogZs[s] = emit_logZ(statg2[:], 4, (s % 2) * 2)
                            emit_out(s)

                for rep in range(reps):
                    emit_rep(rep)

    nc.compile()
    return nc, sorted(probes)


_NC_CACHE = {}


def kernel(**inputs):
    """Full-input entry point: returns logp [B, T, V1] float32."""
    from concourse.bass_utils import run_bass_kernel_spmd
    in_maps = host_prep(inputs)
    if "nc" not in _NC_CACHE:
        _NC_CACHE["nc"], _ = build(T, (), reps=1)
    nc = _NC_CACHE["nc"]
    res = run_bass_kernel_spmd(nc, in_maps, list(range(NC)))
    outs = [res.results[c]["logp"] for c in range(NC)]
    full = np.concatenate(outs, axis=1)[:, :V1]          # [T*B, V1]
    logp = full.reshape(T, B, V1).transpose(1, 0, 2)
    return np.ascontiguousarray(logp.astype(np.float32))

